# revision 12
# baseline (speedup 1.0000x reference)
"""Trainium2 Bass kernel for nn_ContrastiveLoss (4x1000x2048 features, 16 classes).

Sharding: 8 cores = (4 samples) x (2 row-halves of the 1000x1000 similarity
block). Each core computes a [1000 cols x 500 rows] transposed Gram block with
the sample's columns permuted so the core's own 500 rows sit at positions
0..499 (keeps the SPMD program identical across cores). Class-masked row sums
ride the PE via one-hot matmuls; exp/ln on ScalarE. Each core emits two
scalars (block loss sum, block positive-pair count); the host combines 16
scalars into the final weighted loss.
"""

import math

import numpy as np

import concourse.bacc as bacc
import concourse.bass as bass
import concourse.tile as tile
from concourse import mybir
from concourse.bass_utils import run_bass_kernel_spmd
from concourse import bass_isa

F32 = mybir.dt.float32
F32R = mybir.dt.float32r
AF = mybir.ActivationFunctionType
ALU = mybir.AluOpType

B, N, C = 4, 1000, 2048
NP = 1024  # column dim padded to a multiple of 128 (24 phantom columns)
R = 500  # rows per core
KC = C // 128  # 16 K-chunks
CH = NP // 128  # 8 column chunks (of the transposed-gram partition dim)
M16 = 16  # one-hot classes
T = 0.07
INV_T = 1.0 / T
EXP_INV_T = math.exp(INV_T)

_CACHE = {}


def _build_program():
    nc = bacc.Bacc(
        "TRN2",
        target_bir_lowering=False,
        debug=False,
        enable_asserts=False,
        num_devices=8,
    )

    ft_d = nc.dram_tensor("ft", [C, NP], F32R, kind="ExternalInput").ap()
    haug_d = nc.dram_tensor("haug", [NP, M16], F32R, kind="ExternalInput").ap()
    hrow_d = nc.dram_tensor("hrow", [16, R], F32R, kind="ExternalInput").ap()
    id_d = nc.dram_tensor("ident", [128, 128], F32R, kind="ExternalInput").ap()
    valid_d = nc.dram_tensor("valid", [128, CH], F32R, kind="ExternalInput").ap()
    out_d = nc.dram_tensor("out", [1, 2], F32, kind="ExternalOutput").ap()


    with tile.TileContext(nc) as tc:
        with (
            tc.tile_pool(name="big", bufs=1) as big,
            tc.tile_pool(name="consts", bufs=1) as consts,
            tc.tile_pool(name="vecs", bufs=1) as vecs,
            tc.tile_pool(name="sq", bufs=3) as sqp,
            tc.tile_pool(name="gc", bufs=3) as gcp,
            tc.tile_pool(name="x2", bufs=3) as x2p,
            tc.tile_pool(name="lt", bufs=3) as ltp,
        ):
            ps0_cm = tc.tile_pool(name="psum0", bufs=1, space="PSUM")
            psv = ps0_cm.__enter__()
            # ---- constants ----
            ones_f = consts.tile([128, 2], F32)
            nc.gpsimd.memset(ones_f[:], 1.0)
            ones_r = consts.tile([128, 2], F32R)
            nc.vector.tensor_copy(ones_r[:], ones_f[:])
            ones12r = consts.tile([1, 2], F32R)
            nc.vector.tensor_copy(ones12r[:], ones_f[0:1, :])
            zbias = consts.tile([128, 1], F32)
            nc.gpsimd.memset(zbias[:], 0.0)
            ebias = consts.tile([1, 1], F32)
            nc.gpsimd.memset(ebias[:], EXP_INV_T)
            ident = consts.tile([128, 128], F32R)
            nc.sync.dma_start(ident[:], id_d[:])
            valid = consts.tile([128, CH], F32R)
            nc.sync.dma_start(valid[:], valid_d[:])
            hrow = consts.tile([16, R], F32R)
            nc.sync.dma_start(hrow[:], hrow_d[:])
            haug = consts.tile([128, CH * M16], F32R)
            nc.sync.dma_start(
                haug[:].rearrange("p (c m) -> p c m", m=M16),
                haug_d.rearrange("(c p) m -> p c m", p=128),
            )

            # ---- load fT [2048, 1000] as 16 K-chunks of [128, 1000] ----
            ftt = big.tile([128, KC * NP], F32R)
            for k in range(KC):
                nc.sync.dma_start(
                    ftt[:, k * NP : (k + 1) * NP], ft_d[k * 128 : (k + 1) * 128, :]
                )

            # ---- phase 0: column sums of squares -> norm scales a_j ----
            ssq_ps = psv.tile([1, NP], F32, tag="ssq")  # 2 banks
            for k in range(KC):
                sq = sqp.tile([128, NP], F32R, tag="sq")
                src = ftt[:, k * NP : (k + 1) * NP]
                if k % 2 == 0:
                    nc.scalar.activation(sq[:], src, AF.Square, bias=zbias[:])
                else:
                    nc.vector.tensor_tensor(sq[:], src, src, ALU.mult)
                for half in range(2):
                    nc.tensor.matmul(
                        ssq_ps[0:1, half * 512 : (half + 1) * 512],
                        ones_r[:, 0:1],
                        sq[:, half * 512 : (half + 1) * 512],
                        start=(k == 0),
                        stop=(k == KC - 1),
                    )
            ssq_sb = vecs.tile([1, NP], F32R)
            nc.vector.tensor_copy(ssq_sb[:], ssq_ps[:])

            # row -> col layout: sscol[p, c] = ssq[c*128 + p]
            sscol_ps = psv.tile([128, 2 * CH], F32, tag="sscol")
            for c in range(CH):
                nc.tensor.matmul(
                    sscol_ps[:, 2 * c : 2 * c + 2],
                    ssq_sb[0:1, c * 128 : (c + 1) * 128],
                    ones12r[:],
                    start=True,
                    stop=True,
                )
            # a = rsqrt(T * ssq), one Newton step for accuracy
            xcol = vecs.tile([128, CH], F32)
            nc.vector.tensor_scalar(
                xcol[:],
                sscol_ps[:].rearrange("p (c two) -> p two c", two=2)[:, 0],
                T,
                None,
                ALU.mult,
            )
            rec = vecs.tile([128, CH], F32)
            nc.vector.reciprocal(rec[:], xcol[:])
            y0 = vecs.tile([128, CH], F32)
            nc.scalar.activation(y0[:], rec[:], AF.Sqrt, bias=zbias[:])
            y2 = vecs.tile([128, CH], F32)
            nc.vector.tensor_tensor(y2[:], y0[:], y0[:], ALU.mult)
            xy2 = vecs.tile([128, CH], F32)
            nc.vector.tensor_tensor(xy2[:], y2[:], xcol[:], ALU.mult)
            w15 = vecs.tile([128, CH], F32)
            nc.vector.tensor_scalar(w15[:], xy2[:], -0.5, 1.5, ALU.mult, ALU.add)
            y1 = vecs.tile([128, CH], F32)
            nc.vector.tensor_tensor(y1[:], y0[:], w15[:], ALU.mult)
            y1sq = vecs.tile([128, CH], F32)
            nc.vector.tensor_tensor(y1sq[:], y1[:], y1[:], ALU.mult)
            xy1 = vecs.tile([128, CH], F32)
            nc.vector.tensor_tensor(xy1[:], y1sq[:], xcol[:], ALU.mult)
            w15b = vecs.tile([128, CH], F32)
            nc.vector.tensor_scalar(w15b[:], xy1[:], -0.5, 1.5, ALU.mult, ALU.add)
            acol = vecs.tile([128, CH], F32)
            nc.vector.tensor_tensor(acol[:], y1[:], w15b[:], ALU.mult)
            acol_r = vecs.tile([128, CH], F32R)
            nc.vector.tensor_copy(acol_r[:], acol[:])

            # col -> row: arow[0, c*128+p] = acol[p, c]
            arow_ps = psv.tile([1, NP], F32, tag="arow")
            for c in range(CH):
                nc.tensor.matmul(
                    arow_ps[0:1, c * 128 : (c + 1) * 128],
                    acol_r[:, c : c + 1],
                    ident[:],
                    start=True,
                    stop=True,
                )
            arow_sb = vecs.tile([1, NP], F32)
            nc.vector.tensor_copy(arow_sb[:], arow_ps[:])
            abc = big.tile([128, R], F32)
            nc.gpsimd.partition_broadcast(abc[:], arow_sb[0:1, 0:R])

            # a_j-weighted one-hots for the sim-sum matmul
            hauga = consts.tile([128, CH * M16], F32R)
            for c in range(CH):
                sl = slice(c * M16, (c + 1) * M16)
                nc.vector.tensor_scalar(
                    hauga[:, sl], haug[:, sl], acol[:, c : c + 1], None, ALU.mult
                )

            # class counts -> pvec_i = count[class(i)]
            cnt_ps = psv.tile([M16, 2], F32, tag="cnt")
            for c in range(CH):
                nc.tensor.matmul(
                    cnt_ps[:],
                    haug[:, c * M16 : (c + 1) * M16],
                    ones_r[:],
                    start=(c == 0),
                    stop=(c == CH - 1),
                )
            cnt_sb = vecs.tile([M16, 2], F32R)
            nc.vector.tensor_copy(cnt_sb[:], cnt_ps[:])
            pvec_ps = psv.tile([1, R], F32, tag="vec")
            nc.tensor.matmul(
                pvec_ps[:], cnt_sb[0:16, 0:1], hrow[:], start=True, stop=True
            )
            pvec_sb = vecs.tile([1, R], F32)
            nc.vector.tensor_copy(pvec_sb[:], pvec_ps[:])

            ps0_cm.__exit__(None, None, None)
            psg_cm = tc.tile_pool(name="psum_g", bufs=2, space="PSUM")
            psg = psg_cm.__enter__()
            psy_cm = tc.tile_pool(name="psum_y", bufs=1, space="PSUM")
            psy = psy_cm.__enter__()
            pse_cm = tc.tile_pool(name="psum_e", bufs=2, space="PSUM")
            pse = pse_cm.__enter__()

            # ---- phase A: Gram chunks, E = exp(sim), masked-sum matmuls ----
            ye_ps = psy.tile([M16, R], F32, tag="ye")
            sume_ps = psy.tile([1, R], F32, tag="sume")
            yg_ps = psy.tile([M16, R], F32, tag="yg")
            e_all = big.tile([128, CH * R], F32R)
            for c in range(CH):
                g_ps = psg.tile([128, R], F32, tag="g")
                for k in range(KC):
                    nc.tensor.matmul(
                        g_ps[:],
                        ftt[:, k * NP + c * 128 : k * NP + (c + 1) * 128],
                        ftt[:, k * NP : k * NP + R],
                        start=(k == 0),
                        stop=(k == KC - 1),
                    )
                gc1 = gcp.tile([128, R], F32R, tag="gc1")
                nc.vector.tensor_tensor(gc1[:], g_ps[:], abc[:], ALU.mult)
                esl = e_all[:, c * R : (c + 1) * R]
                nc.scalar.activation(
                    esl, gc1[:], AF.Exp, bias=zbias[:], scale=acol[:, c : c + 1]
                )
                nc.tensor.matmul(
                    ye_ps[:],
                    haug[:, c * M16 : (c + 1) * M16],
                    esl,
                    start=(c == 0),
                    stop=(c == CH - 1),
                )
                nc.tensor.matmul(
                    sume_ps[:],
                    valid[:, c : c + 1],
                    esl,
                    start=(c == 0),
                    stop=(c == CH - 1),
                )
                nc.tensor.matmul(
                    yg_ps[:],
                    hauga[:, c * M16 : (c + 1) * M16],
                    gc1[:],
                    start=(c == 0),
                    stop=(c == CH - 1),
                )

            # ---- phase A epilogue: r_i = sumE - same-class sumE ----
            ze = vecs.tile([16, R], F32)
            nc.vector.tensor_tensor(ze[:], ye_ps[0:16, :], hrow[:], ALU.mult)
            zer = vecs.tile([16, R], F32)
            nc.gpsimd.partition_all_reduce(zer[:], ze[:], 16, bass_isa.ReduceOp.add)
            sume_sb = vecs.tile([1, R], F32)
            nc.vector.tensor_copy(sume_sb[:], sume_ps[:])
            r_sb = vecs.tile([1, R], F32)
            nc.vector.tensor_tensor(r_sb[:], sume_sb[:], zer[0:1, :], ALU.subtract)
            rb = big.tile([128, R], F32)
            nc.gpsimd.partition_broadcast(rb[:], r_sb[:])

            ln1p_r = vecs.tile([1, R], F32)
            nc.scalar.activation(ln1p_r[:], r_sb[:], AF.Ln, bias=1.0)
            l_diag = vecs.tile([1, R], F32)
            nc.scalar.activation(l_diag[:], r_sb[:], AF.Ln, bias=ebias[:])
            t4 = vecs.tile([1, R], F32)
            nc.vector.tensor_scalar(t4[:], pvec_sb[:], -1.0, 1001.0, ALU.mult, ALU.add)
            t5 = vecs.tile([1, R], F32)
            nc.vector.tensor_tensor(t5[:], t4[:], ln1p_r[:], ALU.mult)

            # ---- phase B: L = ln(E + r_i), masked-sum matmul ----
            yl_ps = psy.tile([M16, R], F32, tag="yl")
            for c in range(CH):
                x2 = x2p.tile([128, R], F32, tag="x2")
                nc.vector.tensor_tensor(
                    x2[:], e_all[:, c * R : (c + 1) * R], rb[:], ALU.add
                )
                lt = ltp.tile([128, R], F32R, tag="lt")
                nc.scalar.activation(lt[:], x2[:], AF.Ln, bias=zbias[:])
                nc.tensor.matmul(
                    yl_ps[:],
                    haug[:, c * M16 : (c + 1) * M16],
                    lt[:],
                    start=(c == 0),
                    stop=(c == CH - 1),
                )

            # ---- phase B epilogue: assemble per-row losses ----
            zl = vecs.tile([16, R], F32)
            nc.vector.tensor_tensor(zl[:], yl_ps[0:16, :], hrow[:], ALU.mult)
            zlr = vecs.tile([16, R], F32)
            nc.gpsimd.partition_all_reduce(zlr[:], zl[:], 16, bass_isa.ReduceOp.add)
            zg = vecs.tile([16, R], F32)
            nc.vector.tensor_tensor(zg[:], yg_ps[0:16, :], hrow[:], ALU.mult)
            zgr = vecs.tile([16, R], F32)
            nc.gpsimd.partition_all_reduce(zgr[:], zg[:], 16, bass_isa.ReduceOp.add)

            # row_loss = (sumposL - l_diag) - sumposSim + invT + (1001-pvec)*ln1p_r
            posl = vecs.tile([1, R], F32)
            nc.vector.tensor_tensor(posl[:], zlr[0:1, :], l_diag[:], ALU.subtract)
            acc = vecs.tile([1, R], F32)
            nc.vector.tensor_tensor(acc[:], posl[:], zgr[0:1, :], ALU.subtract)
            acc2 = vecs.tile([1, R], F32)
            nc.vector.tensor_scalar(acc2[:], acc[:], INV_T, None, ALU.add)
            rowl = vecs.tile([1, R], F32)
            nc.vector.tensor_tensor(rowl[:], acc2[:], t5[:], ALU.add)

            outv = vecs.tile([1, 2], F32)
            nc.vector.tensor_reduce(
                outv[0:1, 0:1], rowl[:], mybir.AxisListType.X, ALU.add
            )
            pred = vecs.tile([1, 1], F32)
            nc.vector.tensor_reduce(pred[:], pvec_sb[:], mybir.AxisListType.X, ALU.add)
            nc.vector.tensor_scalar(
                outv[0:1, 1:2], pred[:], 1.0, -float(R), ALU.mult, ALU.add
            )
            nc.sync.dma_start(out_d[:], outv[:])
            pse_cm.__exit__(None, None, None)
            psy_cm.__exit__(None, None, None)
            psg_cm.__exit__(None, None, None)

    nc.compile()
    return nc


def _get_program():
    if "nc" not in _CACHE:
        _CACHE["nc"] = _build_program()
    return _CACHE["nc"]


def _make_in_maps(features, target):
    f = np.asarray(features, dtype=np.float32)
    t = np.asarray(target)
    ident = np.eye(128, dtype=np.float32)
    valid = (np.arange(NP) < N).astype(np.float32).reshape(CH, 128).T.copy()
    in_maps = []
    for core in range(8):
        s, h = core // 2, core % 2
        ftp = np.zeros((C, NP), np.float32)
        if h == 0:
            ftp[:, :N] = f[s].T
            tp = np.asarray(t[s])
        else:
            ft = f[s].T
            ftp[:, :R] = ft[:, R:]
            ftp[:, R:N] = ft[:, :R]
            tp = np.concatenate([t[s][R:], t[s][:R]])
        ftp[0, N:] = 1.0  # phantom cols: unit vector -> finite norms/sims
        haug = np.zeros((NP, M16), np.float32)
        haug[np.arange(N), tp.astype(np.int64)] = 1.0
        hrow = np.ascontiguousarray(haug[:R, :16].T)
        in_maps.append({"ft": ftp, "haug": haug, "hrow": hrow, "ident": ident, "valid": valid})
    return in_maps


def _combine(results):
    outs = np.array([r["out"][0] for r in results], dtype=np.float64)  # [8, 2]
    loss_blk = outs[:, 0].reshape(B, 2).sum(axis=1)
    pos_blk = outs[:, 1].reshape(B, 2).sum(axis=1)
    losses = loss_blk / (pos_blk + 1e-6)
    valid = pos_blk > 0
    num = valid.sum()
    if num > 0:
        res = 0.1 * np.where(valid, losses, 0.0).sum() / num
    else:
        res = 0.1 * 0.1
    return np.float32(res)


def kernel(features, target, _trace=False):
    nc = _get_program()
    in_maps = _make_in_maps(features, target)
    out = run_bass_kernel_spmd(nc, in_maps, list(range(8)), trace=_trace)
    result = _combine(out.results)
    if _trace:
        _CACHE["last_exec_time_ns"] = out.exec_time_ns
        _CACHE["last_profile"] = out
    return result


# revision 14
# speedup vs baseline: 1.4003x; 1.4003x over previous
"""Trainium2 Bass kernel for nn_ContrastiveLoss (4x1000x2048 features, 16 classes).

Sharding: 8 cores = (4 samples) x (2 row-halves of the 1000x1000 similarity
block). Each core computes a [1024 cols x 500 rows] transposed Gram block (24
phantom columns pad 1000 -> 1024) with the sample's columns permuted so the
core's own 500 rows sit at positions 0..499 (keeps the SPMD program identical
across cores). The Gram runs in bf16 on the PE; class-masked row sums ride the
PE via one-hot matmuls (ones column first -> row 0 of the Y accumulator is the
phantom-free total); exp/ln on ScalarE. Each core emits two scalars (block
loss sum, block positive-pair count); the host combines 16 scalars.
"""

import math

import numpy as np
import ml_dtypes

import concourse.bacc as bacc
import concourse.bass as bass
import concourse.tile as tile
from concourse import mybir
from concourse.bass_utils import run_bass_kernel_spmd

F32 = mybir.dt.float32
F32R = mybir.dt.float32r
BF16 = mybir.dt.bfloat16
AF = mybir.ActivationFunctionType
ALU = mybir.AluOpType

B, N, C = 4, 1000, 2048
NP = 1024  # column dim padded to a multiple of 128 (24 phantom columns)
R = 500  # rows per core
KC = C // 128  # 16 K-chunks
CH = NP // 128  # 8 column chunks (of the transposed-gram partition dim)
M17 = 17  # ones column + 16 one-hot classes
T = 0.07
INV_T = 1.0 / T
EXP_INV_T = math.exp(INV_T)

N_EARLY = 2  # gram chunks computed K-interleaved during the ft DMA window

_CACHE = {}


def _build_program():
    nc = bacc.Bacc(
        "TRN2",
        target_bir_lowering=False,
        debug=False,
        enable_asserts=False,
        num_devices=8,
    )

    ft_d = nc.dram_tensor("ft", [C, NP], BF16, kind="ExternalInput").ap()
    haug_d = nc.dram_tensor("haug", [NP, M17], F32R, kind="ExternalInput").ap()
    hrow_d = nc.dram_tensor("hrow", [M17, R], F32, kind="ExternalInput").ap()
    id_d = nc.dram_tensor("ident", [128, 128], F32R, kind="ExternalInput").ap()
    out_d = nc.dram_tensor("out", [1, 2], F32, kind="ExternalOutput").ap()

    with tile.TileContext(nc) as tc:
        with (
            tc.tile_pool(name="big", bufs=1) as big,
            tc.tile_pool(name="consts", bufs=1) as consts,
            tc.tile_pool(name="vecs", bufs=1) as vecs,
            tc.tile_pool(name="sq", bufs=3) as sqp,
            tc.tile_pool(name="gc", bufs=3) as gcp,
            tc.tile_pool(name="x2", bufs=3) as x2p,
            tc.tile_pool(name="lt", bufs=3) as ltp,
        ):
            # ---- load fT [2048, 1024] bf16 as 16 K-chunks of [128, 1024] ----
            ftt = big.tile([128, KC * NP], BF16)
            for k in range(KC):
                nc.sync.dma_start(
                    ftt[:, k * NP : (k + 1) * NP], ft_d[k * 128 : (k + 1) * 128, :]
                )
            # small inputs after the bulk loads on the same queue
            ident = consts.tile([128, 128], F32R)
            nc.sync.dma_start(ident[:], id_d[:])
            hrow = consts.tile([M17, R], F32)
            nc.sync.dma_start(hrow[:], hrow_d[:])
            haug = consts.tile([128, CH * M17], F32R)
            nc.sync.dma_start(
                haug[:].rearrange("p (c m) -> p c m", m=M17),
                haug_d.rearrange("(c p) m -> p c m", p=128),
            )

            # ---- constants ----
            ones_f = consts.tile([128, 2], F32)
            nc.gpsimd.memset(ones_f[:], 1.0)
            ones_r = consts.tile([128, 2], F32R)
            nc.vector.tensor_copy(ones_r[:], ones_f[:])
            ones12r = consts.tile([1, 2], F32R)
            nc.vector.tensor_copy(ones12r[:], ones_f[0:1, :])
            ones17f = consts.tile([M17, 1], F32)
            nc.gpsimd.memset(ones17f[:], 1.0)
            zbias = consts.tile([128, 1], F32)
            nc.gpsimd.memset(zbias[:], 0.0)
            ebias = consts.tile([1, 1], F32)
            nc.gpsimd.memset(ebias[:], EXP_INV_T)

            ps_ge_cm = tc.tile_pool(name="psum_ge", bufs=1, space="PSUM")
            ps_ge = ps_ge_cm.__enter__()
            ps0_cm = tc.tile_pool(name="psum0", bufs=1, space="PSUM")
            ps0 = ps0_cm.__enter__()

            # ---- phase 0: squares -> column ssq; early gram chunks ----
            ssq_ps = ps0.tile([1, NP], F32, tag="wide2")  # 2 banks
            ge_tiles = [
                ps_ge.tile([128, R], F32, tag=f"g{c}", name=f"ge{c}")
                for c in range(N_EARLY)
            ]
            for k in range(KC):
                sq = sqp.tile([128, NP], F32R, tag="sq")
                src = ftt[:, k * NP : (k + 1) * NP]
                if k % 2 == 0:
                    nc.scalar.activation(sq[:], src, AF.Square, bias=zbias[:])
                else:
                    nc.vector.tensor_tensor(sq[:], src, src, ALU.mult)
                for half in range(2):
                    nc.tensor.matmul(
                        ssq_ps[0:1, half * 512 : (half + 1) * 512],
                        ones_r[:, 0:1],
                        sq[:, half * 512 : (half + 1) * 512],
                        start=(k == 0),
                        stop=(k == KC - 1),
                    )
                for c in range(N_EARLY):
                    nc.tensor.matmul(
                        ge_tiles[c][:],
                        ftt[:, k * NP + c * 128 : k * NP + (c + 1) * 128],
                        ftt[:, k * NP : k * NP + R],
                        start=(k == 0),
                        stop=(k == KC - 1),
                    )

            # ---- a_j = rsqrt(T * ssq), two Newton steps ----
            ssq_sb = vecs.tile([1, NP], F32R)
            nc.vector.tensor_copy(ssq_sb[:], ssq_ps[:])
            sscol_ps = ps0.tile([128, 2 * CH], F32, tag="small1")
            for c in range(CH):
                nc.tensor.matmul(
                    sscol_ps[:, 2 * c : 2 * c + 2],
                    ssq_sb[0:1, c * 128 : (c + 1) * 128],
                    ones12r[:],
                    start=True,
                    stop=True,
                )
            xcol = vecs.tile([128, CH], F32)
            nc.vector.tensor_scalar(
                xcol[:],
                sscol_ps[:].rearrange("p (c two) -> p two c", two=2)[:, 0],
                T,
                None,
                ALU.mult,
            )
            rec = vecs.tile([128, CH], F32)
            nc.vector.reciprocal(rec[:], xcol[:])
            y0 = vecs.tile([128, CH], F32)
            nc.scalar.activation(y0[:], rec[:], AF.Sqrt, bias=zbias[:])
            y2 = vecs.tile([128, CH], F32)
            nc.vector.tensor_tensor(y2[:], y0[:], y0[:], ALU.mult)
            xy2 = vecs.tile([128, CH], F32)
            nc.vector.tensor_tensor(xy2[:], y2[:], xcol[:], ALU.mult)
            w15 = vecs.tile([128, CH], F32)
            nc.vector.tensor_scalar(w15[:], xy2[:], -0.5, 1.5, ALU.mult, ALU.add)
            y1 = vecs.tile([128, CH], F32)
            nc.vector.tensor_tensor(y1[:], y0[:], w15[:], ALU.mult)
            y1sq = vecs.tile([128, CH], F32)
            nc.vector.tensor_tensor(y1sq[:], y1[:], y1[:], ALU.mult)
            xy1 = vecs.tile([128, CH], F32)
            nc.vector.tensor_tensor(xy1[:], y1sq[:], xcol[:], ALU.mult)
            w15b = vecs.tile([128, CH], F32)
            nc.vector.tensor_scalar(w15b[:], xy1[:], -0.5, 1.5, ALU.mult, ALU.add)
            acol = vecs.tile([128, CH], F32)
            nc.vector.tensor_tensor(acol[:], y1[:], w15b[:], ALU.mult)
            acol_r = vecs.tile([128, CH], F32R)
            nc.vector.tensor_copy(acol_r[:], acol[:])

            # col -> row: arow[0, c*128+p] = acol[p, c]
            arow_ps = ps0.tile([1, NP], F32, tag="wide2")
            for c in range(CH):
                nc.tensor.matmul(
                    arow_ps[0:1, c * 128 : (c + 1) * 128],
                    acol_r[:, c : c + 1],
                    ident[:],
                    start=True,
                    stop=True,
                )
            arow_sb = vecs.tile([1, NP], F32)
            nc.vector.tensor_copy(arow_sb[:], arow_ps[:])
            abc = big.tile([128, R], F32)
            nc.gpsimd.partition_broadcast(abc[:], arow_sb[0:1, 0:R])

            # a_j-weighted one-hots for the sim-sum matmul
            hauga = consts.tile([128, CH * M17], F32R)
            for c in range(CH):
                sl = slice(c * M17, (c + 1) * M17)
                nc.vector.tensor_scalar(
                    hauga[:, sl], haug[:, sl], acol[:, c : c + 1], None, ALU.mult
                )

            # class counts -> pvec_i = count[class(i)]
            cnt_ps = ps0.tile([M17, 2], F32, tag="small1")
            for c in range(CH):
                nc.tensor.matmul(
                    cnt_ps[:],
                    haug[:, c * M17 : (c + 1) * M17],
                    ones_r[:],
                    start=(c == 0),
                    stop=(c == CH - 1),
                )
            cnt_sb = vecs.tile([M17, 2], F32)
            nc.vector.tensor_copy(cnt_sb[:], cnt_ps[:])
            pvec_ps = ps0.tile([1, R], F32, tag="small1")
            nc.tensor.matmul(
                pvec_ps[:], cnt_sb[:, 0:1], hrow[:], start=True, stop=True
            )
            pvec_sb = vecs.tile([1, R], F32)
            nc.vector.tensor_copy(pvec_sb[:], pvec_ps[:])
            # block positive count = sum(pvec) - R, computed early
            outv = vecs.tile([1, 2], F32)
            pred = vecs.tile([1, 1], F32)
            nc.vector.tensor_reduce(pred[:], pvec_sb[:], mybir.AxisListType.X, ALU.add)
            nc.vector.tensor_scalar(
                outv[0:1, 1:2], pred[:], 1.0, -float(R), ALU.mult, ALU.add
            )

            ps0_cm.__exit__(None, None, None)
            psy_cm = tc.tile_pool(name="psum_y", bufs=1, space="PSUM")
            psy = psy_cm.__enter__()
            psg_cm = tc.tile_pool(name="psum_g", bufs=3, space="PSUM")
            psg = psg_cm.__enter__()

            # ---- phase A: remaining Gram chunks + E = exp(sim) + Y matmuls ----
            ye_ps = psy.tile([M17, R], F32, tag="ye")
            yg_ps = psy.tile([M17, R], F32, tag="yg")
            e_all = big.tile([128, CH * R], F32R)

            def gram_late(c):
                g = psg.tile([128, R], F32, tag="g")
                for k in range(KC):
                    nc.tensor.matmul(
                        g[:],
                        ftt[:, k * NP + c * 128 : k * NP + (c + 1) * 128],
                        ftt[:, k * NP : k * NP + R],
                        start=(k == 0),
                        stop=(k == KC - 1),
                    )
                return g

            def do_y(c, g_tile, first, last):
                gc1 = gcp.tile([128, R], F32R, tag="gc1")
                nc.vector.tensor_tensor(gc1[:], g_tile, abc[:], ALU.mult)
                esl = e_all[:, c * R : (c + 1) * R]
                nc.scalar.activation(
                    esl, gc1[:], AF.Exp, bias=zbias[:], scale=acol[:, c : c + 1]
                )
                nc.tensor.matmul(
                    ye_ps[:],
                    haug[:, c * M17 : (c + 1) * M17],
                    esl,
                    start=first,
                    stop=last,
                )
                nc.tensor.matmul(
                    yg_ps[:],
                    hauga[:, c * M17 : (c + 1) * M17],
                    gc1[:],
                    start=first,
                    stop=last,
                )

            g_late = {}
            order = []
            for i in range(CH - N_EARLY):
                order.append(("g", N_EARLY + i))
                order.append(("y", i))
            for c in range(CH - N_EARLY, CH):
                order.append(("y", c))
            n_y = 0
            for kind, c in order:
                if kind == "g":
                    g_late[c] = gram_late(c)
                else:
                    gt = ge_tiles[c][:] if c < N_EARLY else g_late[c][:]
                    do_y(c, gt, first=(n_y == 0), last=(n_y == CH - 1))
                    n_y += 1

            psg_cm.__exit__(None, None, None)
            pse_cm = tc.tile_pool(name="psum_e", bufs=2, space="PSUM")
            pse = pse_cm.__enter__()

            # ---- phase A epilogue: r_i = sumE_i - sameclass_sumE_i ----
            ze = vecs.tile([M17, R], F32)
            nc.vector.tensor_tensor(ze[:], ye_ps[:], hrow[:], ALU.mult)
            semm_ps = pse.tile([1, R], F32, tag="vec")
            nc.tensor.matmul(semm_ps[:], ones17f[:], ze[:], start=True, stop=True)
            sume_sb = vecs.tile([1, R], F32)
            nc.vector.tensor_copy(sume_sb[:], ye_ps[0:1, :])
            r_sb = vecs.tile([1, R], F32)
            nc.vector.tensor_tensor(r_sb[:], sume_sb[:], semm_ps[:], ALU.subtract)
            rb = big.tile([128, R], F32)
            nc.gpsimd.partition_broadcast(rb[:], r_sb[:])

            ln1p_r = vecs.tile([1, R], F32)
            nc.scalar.activation(ln1p_r[:], r_sb[:], AF.Ln, bias=1.0)
            l_diag = vecs.tile([1, R], F32)
            nc.scalar.activation(l_diag[:], r_sb[:], AF.Ln, bias=ebias[:])
            t4 = vecs.tile([1, R], F32)
            nc.vector.tensor_scalar(t4[:], pvec_sb[:], -1.0, 1001.0, ALU.mult, ALU.add)
            t5 = vecs.tile([1, R], F32)
            nc.vector.tensor_tensor(t5[:], t4[:], ln1p_r[:], ALU.mult)
            t5b = vecs.tile([1, R], F32)
            nc.vector.tensor_scalar(t5b[:], t5[:], INV_T, None, ALU.add)

            # ---- phase B: L = ln(E + r_i) + masked-sum matmul ----
            yl_ps = psy.tile([M17, R], F32, tag="yl")
            for c in range(CH):
                x2 = x2p.tile([128, R], F32, tag="x2")
                nc.vector.tensor_tensor(
                    x2[:], e_all[:, c * R : (c + 1) * R], rb[:], ALU.add
                )
                lt = ltp.tile([128, R], F32R, tag="lt")
                nc.scalar.activation(lt[:], x2[:], AF.Ln, bias=zbias[:])
                nc.tensor.matmul(
                    yl_ps[:],
                    haug[:, c * M17 : (c + 1) * M17],
                    lt[:],
                    start=(c == 0),
                    stop=(c == CH - 1),
                )

            # ---- phase B epilogue ----
            zl = vecs.tile([M17, R], F32)
            nc.vector.tensor_tensor(zl[:], yl_ps[:], hrow[:], ALU.mult)
            spl_ps = pse.tile([1, R], F32, tag="vec")
            nc.tensor.matmul(spl_ps[:], ones17f[:], zl[:], start=True, stop=True)
            zg = vecs.tile([M17, R], F32)
            nc.vector.tensor_tensor(zg[:], yg_ps[:], hrow[:], ALU.mult)
            sg_ps = pse.tile([1, R], F32, tag="vec")
            nc.tensor.matmul(sg_ps[:], ones17f[:], zg[:], start=True, stop=True)

            # row_loss = (sumposL - l_diag) - sumposSim + (1000-p)*ln1p_r + invT
            posl = vecs.tile([1, R], F32)
            nc.vector.tensor_tensor(posl[:], spl_ps[:], l_diag[:], ALU.subtract)
            acc = vecs.tile([1, R], F32)
            nc.vector.tensor_tensor(acc[:], posl[:], sg_ps[:], ALU.subtract)
            rowl = vecs.tile([1, R], F32)
            nc.vector.tensor_tensor(rowl[:], acc[:], t5b[:], ALU.add)
            nc.vector.tensor_reduce(
                outv[0:1, 0:1], rowl[:], mybir.AxisListType.X, ALU.add
            )
            nc.sync.dma_start(out_d[:], outv[:])

            pse_cm.__exit__(None, None, None)
            psy_cm.__exit__(None, None, None)
            ps_ge_cm.__exit__(None, None, None)

    nc.compile()
    return nc


def _get_program():
    if "nc" not in _CACHE:
        _CACHE["nc"] = _build_program()
    return _CACHE["nc"]


def _make_in_maps(features, target):
    f = np.asarray(features, dtype=np.float32)
    t = np.asarray(target)
    ident = np.eye(128, dtype=np.float32)
    in_maps = []
    for core in range(8):
        s, h = core // 2, core % 2
        ftp = np.zeros((C, NP), np.float32)
        if h == 0:
            ftp[:, :N] = f[s].T
            tp = np.asarray(t[s])
        else:
            ft = f[s].T
            ftp[:, :R] = ft[:, R:]
            ftp[:, R:N] = ft[:, :R]
            tp = np.concatenate([t[s][R:], t[s][:R]])
        ftp[0, N:] = 1.0  # phantom cols: unit vector -> finite norms/sims
        ftp = ftp.astype(ml_dtypes.bfloat16)
        haug = np.zeros((NP, M17), np.float32)
        haug[:N, 0] = 1.0  # ones column (real cols only)
        haug[np.arange(N), 1 + tp.astype(np.int64)] = 1.0
        hrow = np.zeros((M17, R), np.float32)
        hrow[1:, :] = haug[:R, 1:].T  # row 0 stays zero
        in_maps.append({"ft": ftp, "haug": haug, "hrow": hrow, "ident": ident})
    return in_maps


def _combine(results):
    outs = np.array([r["out"][0] for r in results], dtype=np.float64)  # [8, 2]
    loss_blk = outs[:, 0].reshape(B, 2).sum(axis=1)
    pos_blk = outs[:, 1].reshape(B, 2).sum(axis=1)
    losses = loss_blk / (pos_blk + 1e-6)
    valid = pos_blk > 0
    num = valid.sum()
    if num > 0:
        res = 0.1 * np.where(valid, losses, 0.0).sum() / num
    else:
        res = 0.1 * 0.1
    return np.float32(res)


def kernel(features, target, _trace=False):
    nc = _get_program()
    in_maps = _make_in_maps(features, target)
    out = run_bass_kernel_spmd(nc, in_maps, list(range(8)), trace=_trace)
    result = _combine(out.results)
    if _trace:
        _CACHE["last_exec_time_ns"] = out.exec_time_ns
        _CACHE["last_profile"] = out
    return result
